# revision 32
# baseline (speedup 1.0000x reference)
"""AttnBlock (GroupNorm + single-head spatial self-attention + residual) on
8 Trainium2 NeuronCores — fp8 DoubleRow edition.

Sharding: batch (4) x query-half (2) -> 8 independent shards, one per core.
The host rolls the flattened spatial axis by 2048 for odd cores so each
core's queries are the first 2048 columns of its local x; K/V see all 4096.

Host preprocessing (per batch, standard norm/weight folding):
  - GroupNorm stats -> alpha/beta folded into the conv weights:
    A = diag(alpha) (Wq^T Wk) diag(alpha)  (scores bilinear form, the
    M-trick: per-query affine cancels under softmax, per-key O(mean) term
    dropped), Wsv = diag(alpha) Wv^T Wp^T (V conv with the output
    projection folded in), bpp = bp + Wp(bv + Wv beta).
  - The two 1x1 convs are evaluated host-side in fp32 and shipped as
    single-quantized fp8e4 operands scaled x16: k_sb = fp8(16 A x) and
    vT = fp8(16 x^T Wsv); queries ship twice (fp8e4 for the score matmul,
    bf16 for the residual).

Device pipeline per core — pure attention, every matmul fp8 DoubleRow
(contraction 256 in one pass, 2 fp8 MACs/cell/cycle):
  4 query chunks of 512, flat 2-pair software pipeline across chunks:
    scores st[j,q] = k_sb^T xq8 (pair tiles in 2 PSUM banks),
    P = exp(st/256 - shift) -> fp8e5 pair tiles (ScalarE, per-partition
    bias carries the shift; e5m2 makes overflow impossible),
    PV: a[c,q] += vT pair^T P pair, and Z accumulated on the PE with a
    [128,2,1] ones DoubleRow matmul per pair (partition reductions are
    ~6x cheaper on the PE than on DVE/GpSimd for fp8 operands).
  Epilogue per chunk (DVE+GpSimd): a * 1/(16Z) + (x + bpp), streamed out.

Steady state is ScalarE-bound: exp of a [128,1024] pair costs
(1024+352)/1.2GHz = 1147ns vs 5x215ns = 1075ns of PE matmul per pair.
"""
import numpy as np

B, C, H, W = 4, 256, 64, 64
N = H * W            # 4096 spatial positions
NQ = N // 2          # 2048 queries per core
P = 128              # partitions
CT = C // P          # 2 channel tiles
NUM_GROUPS = 8
EPS = 1e-5
WSCALE = 16.0        # fp8 operand prescale
EXP_SCALE = 1.0 / 256.0   # score descale: 1/16 (attn) * 1/16 (WSCALE)
WARM_MMS = 3

_CACHED = {}


def _build():
    import concourse.bass as bass
    import concourse.mybir as mybir
    import concourse.tile as tile
    from concourse import bacc

    dt = mybir.dt
    AF = mybir.ActivationFunctionType
    DR = mybir.MatmulPerfMode.DoubleRow

    nc = bacc.Bacc("TRN2", debug=False, num_devices=8)

    ksb_d = nc.dram_tensor("ksb", [P, CT * N], dt.float8e4, kind="ExternalInput")
    vt_d = nc.dram_tensor("vt", [P, 32 * C], dt.float8e4, kind="ExternalInput")
    xs_d = nc.dram_tensor("xs", [P, CT * NQ], dt.float8e4, kind="ExternalInput")
    xq_d = nc.dram_tensor("xq", [P, CT * NQ], dt.bfloat16, kind="ExternalInput")
    aux_d = nc.dram_tensor("aux", [P, 8], dt.float32, kind="ExternalInput")
    out_d = nc.dram_tensor("out", [C, NQ], dt.float32, kind="ExternalOutput")

    out_ap = out_d.ap().rearrange("(t p) n -> p t n", p=P)

    with tile.TileContext(nc) as tc:
        with (
            nc.allow_low_precision(reason="fp8 attention is intentional"),
            tc.tile_pool(name="persist", bufs=1) as pe_,
            tc.tile_pool(name="pt", bufs=6) as ptp,
            tc.tile_pool(name="tmp", bufs=3) as tmp,
            tc.tile_pool(name="mm", bufs=2, space="PSUM") as mmp,
            tc.tile_pool(name="acc", bufs=2, space="PSUM") as accp,
            tc.tile_pool(name="zp", bufs=2, space="PSUM") as zpp,
        ):
            # ---------- DMAs first: queue engines must trigger before any
            # other work lands on them (first transfer has ~3.5us ramp) ----
            ksb = pe_.tile([P, CT, N], dt.float8e4, tag="ksb")
            ksb_flat = ksb.rearrange("p t n -> p (t n)")
            xs8 = pe_.tile([P, CT, NQ], dt.float8e4, tag="xs8")
            xs8_flat = xs8.rearrange("p t n -> p (t n)")
            vT = pe_.tile([P, 32, C], dt.float8e4, tag="vT")
            vT_flat = vT.rearrange("p j c -> p (j c)")
            aux_sb = pe_.tile([P, 8], dt.float32, tag="aux")
            xq_r = pe_.tile([P, CT, NQ], dt.bfloat16, tag="xq")

            # sync queue: k_sb chunks (first consumer, progressive sizes);
            # strided 3D APs move both channel halves in one transfer
            ksb_src = ksb_d.ap().rearrange("p (t n) -> p t n", t=CT)
            kedges = [0, 256, 1024, 2048, 4096]
            for ckb in range(len(kedges) - 1):
                cs = slice(kedges[ckb], kedges[ckb + 1])
                nc.sync.dma_start(ksb[:, :, cs], ksb_src[:, :, cs])
            # scalar queue: chunk-0 score moving cols, then vT (PV consumes
            # pair u at ~11.8 + 1.15*u us — pace the chunks to that)
            xs_src = xs_d.ap().rearrange("p (t n) -> p t n", t=CT)
            nc.scalar.dma_start(xs8[:, :, 0:512], xs_src[:, :, 0:512])
            vedges = [0, 6 * C, 12 * C, 20 * C, 32 * C]
            for ckb in range(len(vedges) - 1):
                fs = slice(vedges[ckb], vedges[ckb + 1])
                nc.scalar.dma_start(vT_flat[:, fs], vt_d.ap()[:, fs])
            nc.scalar.dma_start(xs8[:, :, 512:2048], xs_src[:, :, 512:2048])
            # gpsimd queue: aux (exp bias, needed at first exp), xq
            nc.gpsimd.dma_start(aux_sb, aux_d.ap())
            nc.gpsimd.dma_start(xq_r.rearrange("p t n -> p (t n)"), xq_d.ap())

            bpp = aux_sb[:, 0:2]
            ebias = aux_sb[:, 2:3]  # -shift

            # ---------- constants + PE warm-up ----------
            warm_w = pe_.tile([P, P], dt.bfloat16, tag="warmw")
            nc.vector.memset(warm_w, 0.0)
            warm_x = pe_.tile([P, 512], dt.bfloat16, tag="warmx")
            nc.vector.memset(warm_x, 0.0)
            # pair-dim byte stride must be %16 for DoubleRow ldweights
            ones2_t = pe_.tile([P, 2, 16], dt.float8e5, tag="ones2")
            nc.vector.memset(ones2_t.rearrange("p a b -> p (a b)"), 1.0)
            ones2 = ones2_t[:, :, 0:1]
            ones_row = pe_.tile([1, P], dt.bfloat16, tag="ones1r")
            nc.vector.memset(ones_row, 1.0)
            tjunk = pe_.tile([1, 2], dt.float32, tag="tjunk")
            nc.vector.memset(tjunk, 1.0)
            nc.scalar.activation(tjunk, tjunk, AF.Exp)
            for _ in range(WARM_MMS):
                wps = mmp.tile([P, 2, 512], dt.float32, tag="mm")
                nc.tensor.matmul(wps[:, 0], warm_w, warm_x, start=True, stop=True)

            xb = pe_.tile([P, CT, NQ], dt.bfloat16, tag="xb")

            # ---------- attention: flat pair pipeline across chunks ----------
            NIC = NQ // 512
            NU = 16  # jt pairs per chunk
            pend = {}

            def fin_a(ic):
                isl, a_ps, z_ps = pend[ic]
                acp = tmp.tile([P, CT, 512], dt.float32, tag="acp", name=f"acp{ic}")
                for ch in range(CT):
                    nc.vector.tensor_copy(acp[:, ch], a_ps[ch])
                zc = tmp.tile([1, 512], dt.float32, tag="zc", name=f"zc{ic}")
                nc.vector.tensor_scalar_mul(zc, z_ps, WSCALE)
                zb = tmp.tile([P, 2, 512], dt.float32, tag="zb", name=f"zb{ic}")
                nc.gpsimd.partition_broadcast(zb[:, 0], zc)
                nc.vector.reciprocal_approx_fast(zb[:, 1], zb[:, 0])
                pend[ic] = (isl, acp, zb[:, 1])

            def fin_b(ic):
                isl, acp, zr = pend.pop(ic)
                o_sb = tmp.tile([P, CT, 512], dt.float32, tag="o", name=f"o{ic}")
                for h in range(CT):
                    nc.vector.tensor_mul(o_sb[:, h], acp[:, h], zr)
                    nc.vector.tensor_add(o_sb[:, h], o_sb[:, h], xb[:, h, isl])
                nc.sync.dma_start(out_ap[:, :, isl], o_sb)

            def fin_final(ic):
                # exposed tail: broadcast Z with a K=1 matmul (PE is free),
                # then stream the output in 256-wide pieces
                isl, a_ps, z_ps = pend.pop(ic)
                zc = tmp.tile([1, 512], dt.bfloat16, tag="zcf")
                nc.vector.tensor_scalar_mul(zc, z_ps, WSCALE)
                zb_ps = mmp.tile([P, 2, 512], dt.float32, tag="mm")
                nc.tensor.matmul(zb_ps[:, 0], ones_row, zc, start=True, stop=True)
                zr = tmp.tile([P, 512], dt.float32, tag="zrf")
                nc.vector.reciprocal_approx_fast(zr, zb_ps[:, 0])
                o_sb = tmp.tile([P, CT, 512], dt.float32, tag="o", name="ofin")
                dma_engs = [nc.sync, nc.scalar, nc.sync, nc.scalar]
                for q in range(4):
                    h, hq = q // 2, q % 2
                    qs = slice(hq * 256, (hq + 1) * 256)
                    gsl = slice(isl.start + hq * 256, isl.start + (hq + 1) * 256)
                    oq = o_sb[:, h, qs]
                    # DVE does the PSUM-side muls; gpsimd (SBUF-only) chases
                    # with the residual adds so the two pipelines overlap
                    nc.vector.tensor_mul(oq, a_ps[h][:, qs], zr[:, qs])
                    nc.gpsimd.tensor_add(oq, oq, xb[:, h, gsl])
                    dma_engs[q].dma_start(out_ap[:, h, gsl], oq)

            pairs = [(ic, u) for ic in range(NIC) for u in range(NU)]
            isl_of = lambda ic: slice(ic * 512, (ic + 1) * 512)
            a_ps_of = {}
            z_ps_of = {}
            pts = {}

            def st_exp(ic, u):
                st2 = mmp.tile([P, 2, 512], dt.float32, tag="mm")
                for i in range(2):
                    jt = 2 * u + i
                    nc.tensor.matmul(
                        st2[:, i],
                        ksb[:, :, jt * P : (jt + 1) * P],
                        xs8[:, :, isl_of(ic)],
                        start=True, stop=True, perf_mode=DR,
                    )
                pt2 = ptp.tile([P, 2, 512], dt.float8e5, tag="pt")
                nc.scalar.activation(
                    pt2.rearrange("p a b -> p (a b)"),
                    st2.rearrange("p a b -> p (a b)"),
                    AF.Exp, scale=EXP_SCALE, bias=ebias,
                )
                pts[(ic, u)] = pt2

            st_exp(0, 0)
            st_exp(0, 1)
            for idx, (ic, u) in enumerate(pairs):
                if idx + 2 < len(pairs):
                    st_exp(*pairs[idx + 2])
                if u == 0:
                    a_ps_of[ic] = [
                        accp.tile([P, 512], dt.float32, tag="acc", name=f"acc{ic}_{i}")
                        for i in range(CT)
                    ]
                    z_ps_of[ic] = zpp.tile([1, 512], dt.float32, tag="z", name=f"z{ic}")
                a_ps, z_ps = a_ps_of[ic], z_ps_of[ic]
                pt2 = pts.pop((ic, u))
                # last pair: Z first so the epilogue's Z chain starts under
                # the final PV matmuls
                if u == NU - 1:
                    nc.tensor.matmul(
                        z_ps, ones2, pt2, start=False, stop=True,
                        perf_mode=DR,
                    )
                for ch in range(CT):
                    nc.tensor.matmul(
                        a_ps[ch],
                        vT[:, 2 * u : 2 * u + 2, ch * P : (ch + 1) * P],
                        pt2,
                        start=(u == 0), stop=(u == NU - 1),
                        perf_mode=DR,
                    )
                if u < NU - 1:
                    nc.tensor.matmul(
                        z_ps, ones2, pt2,
                        start=(u == 0), stop=False,
                        perf_mode=DR,
                    )
                if ic == 0 and u == NU - 1:
                    # xb = x + proj-bias (bf16); lands in chunk 1's DVE lull
                    for h in range(CT):
                        for hf in range(2):
                            hs = slice(hf * 1024, (hf + 1) * 1024)
                            nc.vector.tensor_scalar_add(
                                xb[:, h, hs], xq_r[:, h, hs], bpp[:, h : h + 1]
                            )
                if u == NU - 1:
                    pend[ic] = (isl_of(ic), a_ps, z_ps)
                    if ic < NIC - 1:
                        fin_a(ic)
                    if ic > 0:
                        fin_b(ic - 1)
            fin_final(NIC - 1)

    nc.compile()
    return nc


def _get_nc():
    if "nc" not in _CACHED:
        _CACHED["nc"] = _build()
    return _CACHED["nc"]


def kernel(x, gn_scale, gn_bias, wq, bq, wk, bk, wv, bv, wp, bp, _trace=False, _trace_cores=None):
    try:
        import jax
        if jax.config.jax_compilation_cache_dir is None:
            jax.config.update("jax_compilation_cache_dir", "/tmp/attnblock_jax_cache")
            jax.config.update("jax_persistent_cache_min_compile_time_secs", 1.0)
    except Exception:
        pass
    import ml_dtypes
    from concourse.bass_utils import run_bass_kernel_spmd

    bf16 = ml_dtypes.bfloat16
    e4 = ml_dtypes.float8_e4m3
    nc = _get_nc()
    x = np.asarray(x, np.float32).reshape(B, C, N)

    def to_e4(a):
        return np.clip(a, -224.0, 224.0).astype(e4)

    wq64 = np.asarray(wq, np.float64)
    wk64 = np.asarray(wk, np.float64)
    wv64 = np.asarray(wv, np.float64)
    wp64 = np.asarray(wp, np.float64)
    mmat = (wq64.T @ wk64).astype(np.float32)
    wpv = (wv64.T @ wp64.T).astype(np.float32)
    gsc = np.asarray(gn_scale, np.float64)
    gbi = np.asarray(gn_bias, np.float64)
    bv64 = np.asarray(bv, np.float64)
    bp64 = np.asarray(bp, np.float64)

    # per-batch GroupNorm stats -> folded weights -> host conv eval
    cg = C // NUM_GROUPS
    ksb_b, vt_b, aux_b = [], [], []
    rng = np.random.default_rng(0)
    sq = rng.choice(N, 48, replace=False)
    for b in range(B):
        xb32 = x[b]
        xg = xb32.reshape(NUM_GROUPS, cg, N)
        mean = xg.mean(axis=(1, 2), dtype=np.float64)
        var = xg.var(axis=(1, 2), dtype=np.float64)
        rstd = 1.0 / np.sqrt(var + EPS)
        alpha = np.repeat(rstd, cg) * gsc
        beta = gbi - np.repeat(mean * rstd, cg) * gsc
        A = ((alpha[:, None] * mmat) * alpha[None, :]).astype(np.float32)
        Wsv = (alpha[:, None] * wpv).astype(np.float32)
        bpp = bp64 + wp64 @ (bv64 + wv64 @ beta)
        khat = A @ xb32                      # [C, N]
        vhat = (WSCALE * Wsv).T @ xb32       # [C, N] -> transpose later
        smax = float((khat[:, sq].T @ xb32).max()) / 16.0
        shift = max(3.0, smax + 1.0 - 7.0)
        ksb_b.append(to_e4(WSCALE * khat))
        vt_b.append(to_e4(vhat))
        aux = np.zeros((P, 8), np.float32)
        aux[:, 0] = bpp[:P]
        aux[:, 1] = bpp[P:]
        aux[:, 2] = -shift
        aux_b.append(aux)

    in_maps = []
    for core in range(8):
        b, qh = core // 2, core % 2
        roll = (lambda a: a) if qh == 0 else (
            lambda a: np.concatenate([a[:, NQ:], a[:, :NQ]], axis=1)
        )
        xl = roll(x[b])
        kl = roll(ksb_b[b])
        vl = roll(vt_b[b])          # [C, N] fp8
        # k_sb pack [p, t*N + n] with channel = t*128 + p
        kp = np.ascontiguousarray(np.concatenate([kl[:P], kl[P:]], axis=1))
        # vT pack [p, jt*C + c] with n = jt*128 + p
        vp = np.ascontiguousarray(
            vl.T.reshape(32, P, C).transpose(1, 0, 2).reshape(P, 32 * C)
        )
        xsq = np.concatenate([xl[:P, :NQ], xl[P:, :NQ]], axis=1)
        in_maps.append({
            "ksb": kp,
            "vt": vp,
            "xs": to_e4(np.ascontiguousarray(xsq)),
            "xq": np.ascontiguousarray(xsq).astype(bf16),
            "aux": aux_b[b],
        })

    last_err = None
    for attempt in range(3):
        try:
            res = run_bass_kernel_spmd(
                nc, in_maps, core_ids=list(range(8)), trace=_trace,
                trace_cores=_trace_cores,
            )
            break
        except Exception as e:  # transient NRT device faults happen rarely
            last_err = e
            import time as _time

            _time.sleep(2.0 * (attempt + 1))
    else:
        raise last_err
    out = np.empty((B, C, N), np.float32)
    for core in range(8):
        b, qh = core // 2, core % 2
        out[b][:, qh * NQ : (qh + 1) * NQ] = res.results[core]["out"]
    if _trace:
        _CACHED["last_results"] = res
    return out.reshape(B, C, H, W)


# revision 35
# speedup vs baseline: 1.0191x; 1.0191x over previous
"""AttnBlock (GroupNorm + single-head spatial self-attention + residual) on
8 Trainium2 NeuronCores — fp8 DoubleRow edition.

Sharding: batch (4) x query-half (2) -> 8 independent shards, one per core.
The host rolls the flattened spatial axis by 2048 for odd cores so each
core's queries are the first 2048 columns of its local x; K/V see all 4096.

Host preprocessing (per batch, standard norm/weight folding):
  - GroupNorm stats -> alpha/beta folded into the conv weights:
    A = diag(alpha) (Wq^T Wk) diag(alpha)  (scores bilinear form, the
    M-trick: per-query affine cancels under softmax, per-key O(mean) term
    dropped), Wsv = diag(alpha) Wv^T Wp^T (V conv with the output
    projection folded in), bpp = bp + Wp(bv + Wv beta).
  - The two 1x1 convs are evaluated host-side in fp32 and shipped as
    single-quantized fp8e4 operands scaled x16: k_sb = fp8(16 A x) and
    vT = fp8(16 x^T Wsv); queries ship twice (fp8e4 for the score matmul,
    bf16 for the residual).

Device pipeline per core — pure attention, every matmul fp8 DoubleRow
(contraction 256 in one pass, 2 fp8 MACs/cell/cycle):
  4 query chunks of 512, flat 2-pair software pipeline across chunks:
    scores st[j,q] = k_sb^T xq8 (pair tiles in 2 PSUM banks),
    P = exp(st/256 - shift) -> fp8e5 pair tiles (ScalarE, per-partition
    bias carries the shift; e5m2 makes overflow impossible),
    PV: a[c,q] += vT pair^T P pair, and Z accumulated on the PE with a
    [128,2,1] ones DoubleRow matmul per pair (partition reductions are
    ~6x cheaper on the PE than on DVE/GpSimd for fp8 operands).
  Epilogue per chunk (DVE+GpSimd): a * 1/(16Z) + (x + bpp), streamed out.

Steady state is ScalarE-bound: exp of a [128,1024] pair costs
(1024+352)/1.2GHz = 1147ns vs 5x215ns = 1075ns of PE matmul per pair.
"""
import numpy as np

B, C, H, W = 4, 256, 64, 64
N = H * W            # 4096 spatial positions
NQ = N // 2          # 2048 queries per core
P = 128              # partitions
CT = C // P          # 2 channel tiles
NUM_GROUPS = 8
EPS = 1e-5
WSCALE = 16.0        # fp8 operand prescale
EXP_SCALE = 1.0 / 256.0   # score descale: 1/16 (attn) * 1/16 (WSCALE)
WARM_MMS = 3

_CACHED = {}


def _build():
    import concourse.bass as bass
    import concourse.mybir as mybir
    import concourse.tile as tile
    from concourse import bacc

    dt = mybir.dt
    AF = mybir.ActivationFunctionType
    DR = mybir.MatmulPerfMode.DoubleRow

    nc = bacc.Bacc("TRN2", debug=False, num_devices=8)

    ksb_d = nc.dram_tensor("ksb", [P, CT * N], dt.float8e4, kind="ExternalInput")
    vt_d = nc.dram_tensor("vt", [P, 32 * C], dt.float8e4, kind="ExternalInput")
    xs_d = nc.dram_tensor("xs", [P, CT * NQ], dt.float8e4, kind="ExternalInput")
    xq_d = nc.dram_tensor("xq", [P, CT * NQ], dt.bfloat16, kind="ExternalInput")
    aux_d = nc.dram_tensor("aux", [P, 8], dt.float32, kind="ExternalInput")
    out_d = nc.dram_tensor("out", [C, NQ], dt.float32, kind="ExternalOutput")

    out_ap = out_d.ap().rearrange("(t p) n -> p t n", p=P)

    with tile.TileContext(nc) as tc:
        with (
            nc.allow_low_precision(reason="fp8 attention is intentional"),
            tc.tile_pool(name="persist", bufs=1) as pe_,
            tc.tile_pool(name="pt", bufs=6) as ptp,
            tc.tile_pool(name="tmp", bufs=3) as tmp,
            tc.tile_pool(name="mm", bufs=2, space="PSUM") as mmp,
            tc.tile_pool(name="acc", bufs=2, space="PSUM") as accp,
            tc.tile_pool(name="zp", bufs=2, space="PSUM") as zpp,
        ):
            # ---------- DMAs first: queue engines must trigger before any
            # other work lands on them (first transfer has ~3.5us ramp) ----
            ksb = pe_.tile([P, CT, N], dt.float8e4, tag="ksb")
            ksb_flat = ksb.rearrange("p t n -> p (t n)")
            xs8 = pe_.tile([P, CT, NQ], dt.float8e4, tag="xs8")
            xs8_flat = xs8.rearrange("p t n -> p (t n)")
            vT = pe_.tile([P, 32, C], dt.float8e4, tag="vT")
            vT_flat = vT.rearrange("p j c -> p (j c)")
            aux_sb = pe_.tile([P, 8], dt.float32, tag="aux")
            xq_r = pe_.tile([P, CT, NQ], dt.bfloat16, tag="xq")

            # sync queue: k_sb chunks (first consumer, progressive sizes);
            # strided 3D APs move both channel halves in one transfer
            ksb_src = ksb_d.ap().rearrange("p (t n) -> p t n", t=CT)
            kedges = [0, 256, 1024, 2048, 4096]
            for ckb in range(len(kedges) - 1):
                cs = slice(kedges[ckb], kedges[ckb + 1])
                nc.sync.dma_start(ksb[:, :, cs], ksb_src[:, :, cs])
            # scalar queue: chunk-0 score moving cols, then half of vT (PV
            # consumes pair u at ~11.8 + 1.15*u us); gpsimd takes the other
            # half. xq (residual, needed only at ~35us) is deferred into the
            # chunk loop so it doesn't eat front DMA bandwidth.
            xs_src = xs_d.ap().rearrange("p (t n) -> p t n", t=CT)
            nc.scalar.dma_start(xs8[:, :, 0:512], xs_src[:, :, 0:512])
            nc.gpsimd.dma_start(aux_sb, aux_d.ap())
            vedges = [0, 4 * C, 10 * C, 20 * C, 32 * C]
            for ckb, eng in enumerate((nc.scalar, nc.gpsimd, nc.scalar, nc.gpsimd)):
                fs = slice(vedges[ckb], vedges[ckb + 1])
                eng.dma_start(vT_flat[:, fs], vt_d.ap()[:, fs])
            nc.scalar.dma_start(xs8[:, :, 512:2048], xs_src[:, :, 512:2048])

            def start_xq_dma():
                nc.gpsimd.dma_start(
                    xq_r.rearrange("p t n -> p (t n)"), xq_d.ap()
                )

            bpp = aux_sb[:, 0:2]
            ebias = aux_sb[:, 2:3]  # -shift

            # ---------- constants + PE warm-up ----------
            warm_w = pe_.tile([P, P], dt.bfloat16, tag="warmw")
            nc.vector.memset(warm_w, 0.0)
            warm_x = pe_.tile([P, 512], dt.bfloat16, tag="warmx")
            nc.vector.memset(warm_x, 0.0)
            # pair-dim byte stride must be %16 for DoubleRow ldweights
            ones2_t = pe_.tile([P, 2, 16], dt.float8e5, tag="ones2")
            nc.vector.memset(ones2_t.rearrange("p a b -> p (a b)"), 1.0)
            ones2 = ones2_t[:, :, 0:1]
            ones_row = pe_.tile([1, P], dt.bfloat16, tag="ones1r")
            nc.vector.memset(ones_row, 1.0)
            tjunk = pe_.tile([1, 2], dt.float32, tag="tjunk")
            nc.vector.memset(tjunk, 1.0)
            nc.scalar.activation(tjunk, tjunk, AF.Exp)
            for _ in range(WARM_MMS):
                wps = mmp.tile([P, 2, 512], dt.float32, tag="mm")
                nc.tensor.matmul(wps[:, 0], warm_w, warm_x, start=True, stop=True)

            xb = pe_.tile([P, CT, NQ], dt.bfloat16, tag="xb")

            # ---------- attention: flat pair pipeline across chunks ----------
            NIC = NQ // 512
            NU = 16  # jt pairs per chunk
            pend = {}

            def fin_a(ic):
                isl, a_ps, z_ps = pend[ic]
                acp = tmp.tile([P, CT, 512], dt.float32, tag="acp", name=f"acp{ic}")
                for ch in range(CT):
                    nc.vector.tensor_copy(acp[:, ch], a_ps[ch])
                zc = tmp.tile([1, 512], dt.float32, tag="zc", name=f"zc{ic}")
                nc.vector.tensor_scalar_mul(zc, z_ps, WSCALE)
                zb = tmp.tile([P, 2, 512], dt.float32, tag="zb", name=f"zb{ic}")
                nc.gpsimd.partition_broadcast(zb[:, 0], zc)
                nc.vector.reciprocal_approx_fast(zb[:, 1], zb[:, 0])
                pend[ic] = (isl, acp, zb[:, 1])

            def fin_b(ic):
                isl, acp, zr = pend.pop(ic)
                o_sb = tmp.tile([P, CT, 512], dt.float32, tag="o", name=f"o{ic}")
                for h in range(CT):
                    nc.vector.tensor_mul(o_sb[:, h], acp[:, h], zr)
                    nc.vector.tensor_add(o_sb[:, h], o_sb[:, h], xb[:, h, isl])
                nc.sync.dma_start(out_ap[:, :, isl], o_sb)

            def fin_final(ic):
                # exposed tail: broadcast Z with a K=1 matmul (PE is free),
                # then stream the output in 256-wide pieces
                isl, a_ps, z_ps = pend.pop(ic)
                zc = tmp.tile([1, 512], dt.bfloat16, tag="zcf")
                nc.vector.tensor_scalar_mul(zc, z_ps, WSCALE)
                zb_ps = mmp.tile([P, 2, 512], dt.float32, tag="mm")
                nc.tensor.matmul(zb_ps[:, 0], ones_row, zc, start=True, stop=True)
                zr = tmp.tile([P, 512], dt.float32, tag="zrf")
                nc.vector.reciprocal_approx_fast(zr, zb_ps[:, 0])
                o_sb = tmp.tile([P, CT, 512], dt.float32, tag="o", name="ofin")
                dma_engs = [nc.sync, nc.scalar, nc.sync, nc.scalar]
                for q in range(4):
                    h, hq = q // 2, q % 2
                    qs = slice(hq * 256, (hq + 1) * 256)
                    gsl = slice(isl.start + hq * 256, isl.start + (hq + 1) * 256)
                    oq = o_sb[:, h, qs]
                    # DVE does the PSUM-side muls; gpsimd (SBUF-only) chases
                    # with the residual adds so the two pipelines overlap
                    nc.vector.tensor_mul(oq, a_ps[h][:, qs], zr[:, qs])
                    nc.gpsimd.tensor_add(oq, oq, xb[:, h, gsl])
                    dma_engs[q].dma_start(out_ap[:, h, gsl], oq)

            pairs = [(ic, u) for ic in range(NIC) for u in range(NU)]
            isl_of = lambda ic: slice(ic * 512, (ic + 1) * 512)
            a_ps_of = {}
            z_ps_of = {}
            pts = {}

            def st_exp(ic, u):
                st2 = mmp.tile([P, 2, 512], dt.float32, tag="mm")
                for i in range(2):
                    jt = 2 * u + i
                    nc.tensor.matmul(
                        st2[:, i],
                        ksb[:, :, jt * P : (jt + 1) * P],
                        xs8[:, :, isl_of(ic)],
                        start=True, stop=True, perf_mode=DR,
                    )
                pt2 = ptp.tile([P, 2, 512], dt.float8e5, tag="pt")
                nc.scalar.activation(
                    pt2.rearrange("p a b -> p (a b)"),
                    st2.rearrange("p a b -> p (a b)"),
                    AF.Exp, scale=EXP_SCALE, bias=ebias,
                )
                pts[(ic, u)] = pt2

            st_exp(0, 0)
            st_exp(0, 1)
            for idx, (ic, u) in enumerate(pairs):
                if idx + 2 < len(pairs):
                    st_exp(*pairs[idx + 2])
                if u == 0:
                    a_ps_of[ic] = [
                        accp.tile([P, 512], dt.float32, tag="acc", name=f"acc{ic}_{i}")
                        for i in range(CT)
                    ]
                    z_ps_of[ic] = zpp.tile([1, 512], dt.float32, tag="z", name=f"z{ic}")
                a_ps, z_ps = a_ps_of[ic], z_ps_of[ic]
                pt2 = pts.pop((ic, u))
                # last pair: Z first so the epilogue's Z chain starts under
                # the final PV matmuls
                if u == NU - 1:
                    nc.tensor.matmul(
                        z_ps, ones2, pt2, start=False, stop=True,
                        perf_mode=DR,
                    )
                for ch in range(CT):
                    nc.tensor.matmul(
                        a_ps[ch],
                        vT[:, 2 * u : 2 * u + 2, ch * P : (ch + 1) * P],
                        pt2,
                        start=(u == 0), stop=(u == NU - 1),
                        perf_mode=DR,
                    )
                if u < NU - 1:
                    nc.tensor.matmul(
                        z_ps, ones2, pt2,
                        start=(u == 0), stop=False,
                        perf_mode=DR,
                    )
                if ic == 0 and u == 8:
                    start_xq_dma()
                if ic == 0 and u == NU - 1:
                    # xb = x + proj-bias (bf16); lands in chunk 1's DVE lull
                    for h in range(CT):
                        for hf in range(2):
                            hs = slice(hf * 1024, (hf + 1) * 1024)
                            nc.vector.tensor_scalar_add(
                                xb[:, h, hs], xq_r[:, h, hs], bpp[:, h : h + 1]
                            )
                if u == NU - 1:
                    pend[ic] = (isl_of(ic), a_ps, z_ps)
                    if ic < NIC - 1:
                        fin_a(ic)
                    if ic > 0:
                        fin_b(ic - 1)
            fin_final(NIC - 1)

    nc.compile()
    return nc


def _get_nc():
    if "nc" not in _CACHED:
        _CACHED["nc"] = _build()
    return _CACHED["nc"]


def kernel(x, gn_scale, gn_bias, wq, bq, wk, bk, wv, bv, wp, bp, _trace=False, _trace_cores=None):
    try:
        import jax
        if jax.config.jax_compilation_cache_dir is None:
            jax.config.update("jax_compilation_cache_dir", "/tmp/attnblock_jax_cache")
            jax.config.update("jax_persistent_cache_min_compile_time_secs", 1.0)
    except Exception:
        pass
    import ml_dtypes
    from concourse.bass_utils import run_bass_kernel_spmd

    bf16 = ml_dtypes.bfloat16
    e4 = ml_dtypes.float8_e4m3
    nc = _get_nc()
    x = np.asarray(x, np.float32).reshape(B, C, N)

    def to_e4(a):
        return np.clip(a, -224.0, 224.0).astype(e4)

    wq64 = np.asarray(wq, np.float64)
    wk64 = np.asarray(wk, np.float64)
    wv64 = np.asarray(wv, np.float64)
    wp64 = np.asarray(wp, np.float64)
    mmat = (wq64.T @ wk64).astype(np.float32)
    wpv = (wv64.T @ wp64.T).astype(np.float32)
    gsc = np.asarray(gn_scale, np.float64)
    gbi = np.asarray(gn_bias, np.float64)
    bv64 = np.asarray(bv, np.float64)
    bp64 = np.asarray(bp, np.float64)

    # per-batch GroupNorm stats -> folded weights -> host conv eval
    cg = C // NUM_GROUPS
    ksb_b, vt_b, aux_b = [], [], []
    rng = np.random.default_rng(0)
    sq = rng.choice(N, 48, replace=False)
    for b in range(B):
        xb32 = x[b]
        xg = xb32.reshape(NUM_GROUPS, cg, N)
        mean = xg.mean(axis=(1, 2), dtype=np.float64)
        var = xg.var(axis=(1, 2), dtype=np.float64)
        rstd = 1.0 / np.sqrt(var + EPS)
        alpha = np.repeat(rstd, cg) * gsc
        beta = gbi - np.repeat(mean * rstd, cg) * gsc
        A = ((alpha[:, None] * mmat) * alpha[None, :]).astype(np.float32)
        Wsv = (alpha[:, None] * wpv).astype(np.float32)
        bpp = bp64 + wp64 @ (bv64 + wv64 @ beta)
        khat = A @ xb32                      # [C, N]
        vhat = (WSCALE * Wsv).T @ xb32       # [C, N] -> transpose later
        smax = float((khat[:, sq].T @ xb32).max()) / 16.0
        shift = max(3.0, smax + 1.0 - 7.0)
        ksb_b.append(to_e4(WSCALE * khat))
        vt_b.append(to_e4(vhat))
        aux = np.zeros((P, 8), np.float32)
        aux[:, 0] = bpp[:P]
        aux[:, 1] = bpp[P:]
        aux[:, 2] = -shift
        aux_b.append(aux)

    in_maps = []
    for core in range(8):
        b, qh = core // 2, core % 2
        roll = (lambda a: a) if qh == 0 else (
            lambda a: np.concatenate([a[:, NQ:], a[:, :NQ]], axis=1)
        )
        xl = roll(x[b])
        kl = roll(ksb_b[b])
        vl = roll(vt_b[b])          # [C, N] fp8
        # k_sb pack [p, t*N + n] with channel = t*128 + p
        kp = np.ascontiguousarray(np.concatenate([kl[:P], kl[P:]], axis=1))
        # vT pack [p, jt*C + c] with n = jt*128 + p
        vp = np.ascontiguousarray(
            vl.T.reshape(32, P, C).transpose(1, 0, 2).reshape(P, 32 * C)
        )
        xsq = np.concatenate([xl[:P, :NQ], xl[P:, :NQ]], axis=1)
        in_maps.append({
            "ksb": kp,
            "vt": vp,
            "xs": to_e4(np.ascontiguousarray(xsq)),
            "xq": np.ascontiguousarray(xsq).astype(bf16),
            "aux": aux_b[b],
        })

    last_err = None
    for attempt in range(3):
        try:
            res = run_bass_kernel_spmd(
                nc, in_maps, core_ids=list(range(8)), trace=_trace,
                trace_cores=_trace_cores,
            )
            break
        except Exception as e:  # transient NRT device faults happen rarely
            last_err = e
            import time as _time

            _time.sleep(2.0 * (attempt + 1))
    else:
        raise last_err
    out = np.empty((B, C, N), np.float32)
    for core in range(8):
        b, qh = core // 2, core % 2
        out[b][:, qh * NQ : (qh + 1) * NQ] = res.results[core]["out"]
    if _trace:
        _CACHED["last_results"] = res
    return out.reshape(B, C, H, W)


# revision 38
# speedup vs baseline: 1.0312x; 1.0119x over previous
"""AttnBlock (GroupNorm + single-head spatial self-attention + residual) on
8 Trainium2 NeuronCores — fp8 DoubleRow edition.

Sharding: batch (4) x query-half (2) -> 8 independent shards, one per core.
The host rolls the flattened spatial axis by 2048 for odd cores so each
core's queries are the first 2048 columns of its local x; K/V see all 4096.

Host preprocessing (per batch, standard norm/weight folding):
  - GroupNorm stats -> alpha/beta folded into the conv weights:
    A = diag(alpha) (Wq^T Wk) diag(alpha)  (scores bilinear form, the
    M-trick: per-query affine cancels under softmax, per-key O(mean) term
    dropped), Wsv = diag(alpha) Wv^T Wp^T (V conv with the output
    projection folded in), bpp = bp + Wp(bv + Wv beta).
  - The two 1x1 convs are evaluated host-side in fp32 and shipped as
    single-quantized fp8e4 operands scaled x16: k_sb = fp8(16 A x) and
    vT = fp8(16 x^T Wsv); queries ship twice (fp8e4 for the score matmul,
    bf16 for the residual).

Device pipeline per core — pure attention, every matmul fp8 DoubleRow
(contraction 256 in one pass, 2 fp8 MACs/cell/cycle):
  4 query chunks of 512, flat 2-pair software pipeline across chunks:
    scores st[j,q] = k_sb^T xq8 (pair tiles in 2 PSUM banks),
    P = exp(st/256 - shift) -> fp8e5 pair tiles (ScalarE, per-partition
    bias carries the shift; e5m2 makes overflow impossible),
    PV: a[c,q] += vT pair^T P pair, and Z accumulated on the PE with a
    [128,2,1] ones DoubleRow matmul per pair (partition reductions are
    ~6x cheaper on the PE than on DVE/GpSimd for fp8 operands).
  Epilogue per chunk (DVE+GpSimd): a * 1/(16Z) + (x + bpp), streamed out.

Steady state is ScalarE-bound: exp of a [128,1024] pair costs
(1024+352)/1.2GHz = 1147ns vs 5x215ns = 1075ns of PE matmul per pair.
"""
import numpy as np

B, C, H, W = 4, 256, 64, 64
N = H * W            # 4096 spatial positions
NQ = N // 2          # 2048 queries per core
P = 128              # partitions
CT = C // P          # 2 channel tiles
NUM_GROUPS = 8
EPS = 1e-5
WSCALE = 16.0        # fp8 operand prescale
EXP_SCALE = 1.0 / 256.0   # score descale: 1/16 (attn) * 1/16 (WSCALE)
WARM_MMS = 3

_CACHED = {}


def _build():
    import concourse.bass as bass
    import concourse.mybir as mybir
    import concourse.tile as tile
    from concourse import bacc

    dt = mybir.dt
    AF = mybir.ActivationFunctionType
    DR = mybir.MatmulPerfMode.DoubleRow

    nc = bacc.Bacc("TRN2", debug=False, num_devices=8)

    ksb_d = nc.dram_tensor("ksb", [P, CT * N], dt.float8e4, kind="ExternalInput")
    vt_d = nc.dram_tensor("vt", [P, 32 * C], dt.float8e4, kind="ExternalInput")
    xs_d = nc.dram_tensor("xs", [P, CT * NQ], dt.float8e4, kind="ExternalInput")
    xq_d = nc.dram_tensor("xq", [P, CT * NQ], dt.bfloat16, kind="ExternalInput")
    aux_d = nc.dram_tensor("aux", [P, 8], dt.float32, kind="ExternalInput")
    out_d = nc.dram_tensor("out", [C, NQ], dt.float32, kind="ExternalOutput")

    out_ap = out_d.ap().rearrange("(t p) n -> p t n", p=P)

    with tile.TileContext(nc) as tc:
        with (
            nc.allow_low_precision(reason="fp8 attention is intentional"),
            tc.tile_pool(name="persist", bufs=1) as pe_,
            tc.tile_pool(name="pt", bufs=6) as ptp,
            tc.tile_pool(name="tmp", bufs=3) as tmp,
            tc.tile_pool(name="mm", bufs=2, space="PSUM") as mmp,
            tc.tile_pool(name="acc", bufs=2, space="PSUM") as accp,
            tc.tile_pool(name="zp", bufs=2, space="PSUM") as zpp,
        ):
            # ---------- DMAs first: queue engines must trigger before any
            # other work lands on them (first transfer has ~3.5us ramp) ----
            # ksb and xs8 are chunk-major [P, ck, t, 512] so every transfer
            # is fully contiguous (strided multi-segment DMAs ran ~2x slower)
            ksb = pe_.tile([P, 8, CT, 512], dt.float8e4, tag="ksb")
            ksb_flat = ksb.rearrange("p k t n -> p (k t n)")
            xs8 = pe_.tile([P, 4, CT, 512], dt.float8e4, tag="xs8")
            xs8_flat = xs8.rearrange("p k t n -> p (k t n)")
            vT = pe_.tile([P, 32, C], dt.float8e4, tag="vT")
            vT_flat = vT.rearrange("p j c -> p (j c)")
            aux_sb = pe_.tile([P, 8], dt.float32, tag="aux")
            xq_r = pe_.tile([P, CT, NQ], dt.bfloat16, tag="xq")

            CKB = CT * 512  # flat elems per ksb/xs8 chunk
            # sync queue: ksb chunks 0-5 progressively (pairs 0-11)
            for lo, hi in ((0, 1), (1, 2), (2, 4), (4, 6)):
                fs = slice(lo * CKB, hi * CKB)
                nc.sync.dma_start(ksb_flat[:, fs], ksb_d.ap()[:, fs])
            # scalar queue: chunk-0 score moving cols, vT for pairs 0-1 and
            # 5-9, the remaining score moving cols
            nc.scalar.dma_start(xs8_flat[:, 0:CKB], xs_d.ap()[:, 0:CKB])
            nc.gpsimd.dma_start(aux_sb, aux_d.ap())
            vedges = [0, 4 * C, 10 * C, 20 * C, 32 * C]
            for ckb, eng in enumerate((nc.scalar, nc.gpsimd, nc.scalar, nc.gpsimd)):
                fs = slice(vedges[ckb], vedges[ckb + 1])
                eng.dma_start(vT_flat[:, fs], vt_d.ap()[:, fs])
            nc.scalar.dma_start(
                xs8_flat[:, CKB : 4 * CKB], xs_d.ap()[:, CKB : 4 * CKB]
            )
            # gpsimd: ksb chunks 6-7 (pairs 12-15, consumed ~26us)
            nc.gpsimd.dma_start(
                ksb_flat[:, 6 * CKB : 8 * CKB], ksb_d.ap()[:, 6 * CKB : 8 * CKB]
            )

            def start_xq_dma():
                nc.gpsimd.dma_start(
                    xq_r.rearrange("p t n -> p (t n)"), xq_d.ap()
                )

            bpp = aux_sb[:, 0:2]
            ebias = aux_sb[:, 2:3]  # -shift

            # ---------- constants + PE warm-up ----------
            warm_w = pe_.tile([P, P], dt.bfloat16, tag="warmw")
            nc.vector.memset(warm_w, 0.0)
            warm_x = pe_.tile([P, 512], dt.bfloat16, tag="warmx")
            nc.vector.memset(warm_x, 0.0)
            # pair-dim byte stride must be %16 for DoubleRow ldweights
            ones2_t = pe_.tile([P, 2, 16], dt.float8e5, tag="ones2")
            nc.vector.memset(ones2_t.rearrange("p a b -> p (a b)"), 1.0)
            ones2 = ones2_t[:, :, 0:1]
            ones_row = pe_.tile([1, P], dt.bfloat16, tag="ones1r")
            nc.vector.memset(ones_row, 1.0)
            tjunk = pe_.tile([1, 2], dt.float32, tag="tjunk")
            nc.vector.memset(tjunk, 1.0)
            nc.scalar.activation(tjunk, tjunk, AF.Exp)
            for _ in range(WARM_MMS):
                wps = mmp.tile([P, 2, 512], dt.float32, tag="mm")
                nc.tensor.matmul(wps[:, 0], warm_w, warm_x, start=True, stop=True)

            xb = pe_.tile([P, CT, NQ], dt.bfloat16, tag="xb")

            # ---------- attention: flat pair pipeline across chunks ----------
            NIC = NQ // 512
            NU = 16  # jt pairs per chunk
            pend = {}

            def fin_a(ic):
                isl, a_ps, z_ps = pend[ic]
                acp = tmp.tile([P, CT, 512], dt.float32, tag="acp", name=f"acp{ic}")
                for ch in range(CT):
                    nc.vector.tensor_copy(acp[:, ch], a_ps[ch])
                zc = tmp.tile([1, 512], dt.float32, tag="zc", name=f"zc{ic}")
                nc.vector.tensor_scalar_mul(zc, z_ps, WSCALE)
                zb = tmp.tile([P, 2, 512], dt.float32, tag="zb", name=f"zb{ic}")
                nc.gpsimd.partition_broadcast(zb[:, 0], zc)
                nc.vector.reciprocal_approx_fast(zb[:, 1], zb[:, 0])
                pend[ic] = (isl, acp, zb[:, 1])

            def fin_b(ic):
                isl, acp, zr = pend.pop(ic)
                o_sb = tmp.tile([P, CT, 512], dt.float32, tag="o", name=f"o{ic}")
                for h in range(CT):
                    nc.vector.tensor_mul(o_sb[:, h], acp[:, h], zr)
                    nc.vector.tensor_add(o_sb[:, h], o_sb[:, h], xb[:, h, isl])
                nc.sync.dma_start(out_ap[:, :, isl], o_sb)

            def fin_final(ic):
                # exposed tail: broadcast Z with a K=1 matmul (PE is free),
                # then stream the output in 256-wide pieces
                isl, a_ps, z_ps = pend.pop(ic)
                zc = tmp.tile([1, 512], dt.bfloat16, tag="zcf")
                nc.vector.tensor_scalar_mul(zc, z_ps, WSCALE)
                zb_ps = mmp.tile([P, 2, 512], dt.float32, tag="mm")
                nc.tensor.matmul(zb_ps[:, 0], ones_row, zc, start=True, stop=True)
                zr = tmp.tile([P, 512], dt.float32, tag="zrf")
                nc.vector.reciprocal_approx_fast(zr, zb_ps[:, 0])
                o_sb = tmp.tile([P, CT, 512], dt.float32, tag="o", name="ofin")
                dma_engs = [nc.sync, nc.scalar, nc.sync, nc.scalar]
                for q in range(4):
                    h, hq = q // 2, q % 2
                    qs = slice(hq * 256, (hq + 1) * 256)
                    gsl = slice(isl.start + hq * 256, isl.start + (hq + 1) * 256)
                    oq = o_sb[:, h, qs]
                    # DVE does the PSUM-side muls; gpsimd (SBUF-only) chases
                    # with the residual adds so the two pipelines overlap
                    nc.vector.tensor_mul(oq, a_ps[h][:, qs], zr[:, qs])
                    nc.gpsimd.tensor_add(oq, oq, xb[:, h, gsl])
                    dma_engs[q].dma_start(out_ap[:, h, gsl], oq)

            pairs = [(ic, u) for ic in range(NIC) for u in range(NU)]
            isl_of = lambda ic: slice(ic * 512, (ic + 1) * 512)
            a_ps_of = {}
            z_ps_of = {}
            pts = {}

            def st_exp(ic, u):
                st2 = mmp.tile([P, 2, 512], dt.float32, tag="mm")
                for i in range(2):
                    jt = 2 * u + i
                    kck, kcol = jt // 4, (jt % 4) * P
                    nc.tensor.matmul(
                        st2[:, i],
                        ksb[:, kck, :, kcol : kcol + P],
                        xs8[:, ic],
                        start=True, stop=True, perf_mode=DR,
                    )
                pt2 = ptp.tile([P, 2, 512], dt.float8e5, tag="pt")
                nc.scalar.activation(
                    pt2.rearrange("p a b -> p (a b)"),
                    st2.rearrange("p a b -> p (a b)"),
                    AF.Exp, scale=EXP_SCALE, bias=ebias,
                )
                pts[(ic, u)] = pt2

            st_exp(0, 0)
            st_exp(0, 1)
            for idx, (ic, u) in enumerate(pairs):
                if idx + 2 < len(pairs):
                    st_exp(*pairs[idx + 2])
                if u == 0:
                    a_ps_of[ic] = [
                        accp.tile([P, 512], dt.float32, tag="acc", name=f"acc{ic}_{i}")
                        for i in range(CT)
                    ]
                    z_ps_of[ic] = zpp.tile([1, 512], dt.float32, tag="z", name=f"z{ic}")
                a_ps, z_ps = a_ps_of[ic], z_ps_of[ic]
                pt2 = pts.pop((ic, u))
                # last pair: Z first so the epilogue's Z chain starts under
                # the final PV matmuls
                if u == NU - 1:
                    nc.tensor.matmul(
                        z_ps, ones2, pt2, start=False, stop=True,
                        perf_mode=DR,
                    )
                for ch in range(CT):
                    nc.tensor.matmul(
                        a_ps[ch],
                        vT[:, 2 * u : 2 * u + 2, ch * P : (ch + 1) * P],
                        pt2,
                        start=(u == 0), stop=(u == NU - 1),
                        perf_mode=DR,
                    )
                if u < NU - 1:
                    nc.tensor.matmul(
                        z_ps, ones2, pt2,
                        start=(u == 0), stop=False,
                        perf_mode=DR,
                    )
                if ic == 0 and u == 8:
                    start_xq_dma()
                if ic == 0 and u == NU - 1:
                    # xb = x + proj-bias (bf16); lands in chunk 1's DVE lull
                    for h in range(CT):
                        for hf in range(2):
                            hs = slice(hf * 1024, (hf + 1) * 1024)
                            nc.vector.tensor_scalar_add(
                                xb[:, h, hs], xq_r[:, h, hs], bpp[:, h : h + 1]
                            )
                if u == NU - 1:
                    pend[ic] = (isl_of(ic), a_ps, z_ps)
                    if ic < NIC - 1:
                        fin_a(ic)
                    if ic > 0:
                        fin_b(ic - 1)
            fin_final(NIC - 1)

    nc.compile()
    return nc


def _get_nc():
    if "nc" not in _CACHED:
        _CACHED["nc"] = _build()
    return _CACHED["nc"]


def kernel(x, gn_scale, gn_bias, wq, bq, wk, bk, wv, bv, wp, bp, _trace=False, _trace_cores=None):
    try:
        import jax
        if jax.config.jax_compilation_cache_dir is None:
            jax.config.update("jax_compilation_cache_dir", "/tmp/attnblock_jax_cache")
            jax.config.update("jax_persistent_cache_min_compile_time_secs", 1.0)
    except Exception:
        pass
    import ml_dtypes
    from concourse.bass_utils import run_bass_kernel_spmd

    bf16 = ml_dtypes.bfloat16
    e4 = ml_dtypes.float8_e4m3
    nc = _get_nc()
    x = np.asarray(x, np.float32).reshape(B, C, N)

    def to_e4(a):
        return np.clip(a, -224.0, 224.0).astype(e4)

    wq64 = np.asarray(wq, np.float64)
    wk64 = np.asarray(wk, np.float64)
    wv64 = np.asarray(wv, np.float64)
    wp64 = np.asarray(wp, np.float64)
    mmat = (wq64.T @ wk64).astype(np.float32)
    wpv = (wv64.T @ wp64.T).astype(np.float32)
    gsc = np.asarray(gn_scale, np.float64)
    gbi = np.asarray(gn_bias, np.float64)
    bv64 = np.asarray(bv, np.float64)
    bp64 = np.asarray(bp, np.float64)

    # per-batch GroupNorm stats -> folded weights -> host conv eval
    cg = C // NUM_GROUPS
    ksb_b, vt_b, aux_b = [], [], []
    rng = np.random.default_rng(0)
    sq = rng.choice(N, 48, replace=False)
    for b in range(B):
        xb32 = x[b]
        xg = xb32.reshape(NUM_GROUPS, cg, N)
        mean = xg.mean(axis=(1, 2), dtype=np.float64)
        var = xg.var(axis=(1, 2), dtype=np.float64)
        rstd = 1.0 / np.sqrt(var + EPS)
        alpha = np.repeat(rstd, cg) * gsc
        beta = gbi - np.repeat(mean * rstd, cg) * gsc
        A = ((alpha[:, None] * mmat) * alpha[None, :]).astype(np.float32)
        Wsv = (alpha[:, None] * wpv).astype(np.float32)
        bpp = bp64 + wp64 @ (bv64 + wv64 @ beta)
        khat = A @ xb32                      # [C, N]
        vhat = (WSCALE * Wsv).T @ xb32       # [C, N] -> transpose later
        smax = float((khat[:, sq].T @ xb32).max()) / 16.0
        shift = max(3.0, smax + 1.0 - 7.0)
        ksb_b.append(to_e4(WSCALE * khat))
        vt_b.append(to_e4(vhat))
        aux = np.zeros((P, 8), np.float32)
        aux[:, 0] = bpp[:P]
        aux[:, 1] = bpp[P:]
        aux[:, 2] = -shift
        aux_b.append(aux)

    in_maps = []
    for core in range(8):
        b, qh = core // 2, core % 2
        roll = (lambda a: a) if qh == 0 else (
            lambda a: np.concatenate([a[:, NQ:], a[:, :NQ]], axis=1)
        )
        xl = roll(x[b])
        kl = roll(ksb_b[b])
        vl = roll(vt_b[b])          # [C, N] fp8
        # k_sb pack chunk-major [p, ck*CT*512 + t*512 + c]
        kp = np.ascontiguousarray(
            kl.reshape(CT, P, 8, 512).transpose(1, 2, 0, 3).reshape(P, -1)
        )
        # vT pack [p, jt*C + c] with n = jt*128 + p
        vp = np.ascontiguousarray(
            vl.T.reshape(32, P, C).transpose(1, 0, 2).reshape(P, 32 * C)
        )
        xsq = xl[:, :NQ]
        xs = np.ascontiguousarray(
            to_e4(xsq).reshape(CT, P, 4, 512).transpose(1, 2, 0, 3).reshape(P, -1)
        )
        in_maps.append({
            "ksb": kp,
            "vt": vp,
            "xs": xs,
            "xq": np.ascontiguousarray(
                np.concatenate([xsq[:P], xsq[P:]], axis=1)
            ).astype(bf16),
            "aux": aux_b[b],
        })

    last_err = None
    for attempt in range(3):
        try:
            res = run_bass_kernel_spmd(
                nc, in_maps, core_ids=list(range(8)), trace=_trace,
                trace_cores=_trace_cores,
            )
            break
        except Exception as e:  # transient NRT device faults happen rarely
            last_err = e
            import time as _time

            _time.sleep(2.0 * (attempt + 1))
    else:
        raise last_err
    out = np.empty((B, C, N), np.float32)
    for core in range(8):
        b, qh = core // 2, core % 2
        out[b][:, qh * NQ : (qh + 1) * NQ] = res.results[core]["out"]
    if _trace:
        _CACHED["last_results"] = res
    return out.reshape(B, C, H, W)


# revision 39
# speedup vs baseline: 1.0417x; 1.0102x over previous
"""AttnBlock (GroupNorm + single-head spatial self-attention + residual) on
8 Trainium2 NeuronCores — fp8 DoubleRow edition.

Sharding: batch (4) x query-half (2) -> 8 independent shards, one per core.
The host rolls the flattened spatial axis by 2048 for odd cores so each
core's queries are the first 2048 columns of its local x; K/V see all 4096.

Host preprocessing (per batch, standard norm/weight folding):
  - GroupNorm stats -> alpha/beta folded into the conv weights:
    A = diag(alpha) (Wq^T Wk) diag(alpha)  (scores bilinear form, the
    M-trick: per-query affine cancels under softmax, per-key O(mean) term
    dropped), Wsv = diag(alpha) Wv^T Wp^T (V conv with the output
    projection folded in), bpp = bp + Wp(bv + Wv beta).
  - The two 1x1 convs are evaluated host-side in fp32 and shipped as
    single-quantized fp8e4 operands scaled x16: k_sb = fp8(16 A x) and
    vT = fp8(16 x^T Wsv); queries ship twice (fp8e4 for the score matmul,
    bf16 for the residual).

Device pipeline per core — pure attention, every matmul fp8 DoubleRow
(contraction 256 in one pass, 2 fp8 MACs/cell/cycle):
  4 query chunks of 512, flat 2-pair software pipeline across chunks:
    scores st[j,q] = k_sb^T xq8 (pair tiles in 2 PSUM banks),
    P = exp(st/256 - shift) -> fp8e5 pair tiles (ScalarE, per-partition
    bias carries the shift; e5m2 makes overflow impossible),
    PV: a[c,q] += vT pair^T P pair, and Z accumulated on the PE with a
    [128,2,1] ones DoubleRow matmul per pair (partition reductions are
    ~6x cheaper on the PE than on DVE/GpSimd for fp8 operands).
  Epilogue per chunk (DVE+GpSimd): a * 1/(16Z) + (x + bpp), streamed out.

Steady state is ScalarE-bound: exp of a [128,1024] pair costs
(1024+352)/1.2GHz = 1147ns vs 5x215ns = 1075ns of PE matmul per pair.
"""
import numpy as np

B, C, H, W = 4, 256, 64, 64
N = H * W            # 4096 spatial positions
NQ = N // 2          # 2048 queries per core
P = 128              # partitions
CT = C // P          # 2 channel tiles
NUM_GROUPS = 8
EPS = 1e-5
WSCALE = 16.0        # fp8 operand prescale
EXP_SCALE = 1.0 / 256.0   # score descale: 1/16 (attn) * 1/16 (WSCALE)
WARM_MMS = 3

_CACHED = {}


def _build():
    import concourse.bass as bass
    import concourse.mybir as mybir
    import concourse.tile as tile
    from concourse import bacc

    dt = mybir.dt
    AF = mybir.ActivationFunctionType
    DR = mybir.MatmulPerfMode.DoubleRow

    nc = bacc.Bacc("TRN2", debug=False, num_devices=8)

    ksb_d = nc.dram_tensor("ksb", [P, CT * N], dt.float8e4, kind="ExternalInput")
    vt_d = nc.dram_tensor("vt", [P, 32 * C], dt.float8e4, kind="ExternalInput")
    xs_d = nc.dram_tensor("xs", [P, CT * NQ], dt.float8e4, kind="ExternalInput")
    xq_d = nc.dram_tensor("xq", [P, CT * NQ], dt.bfloat16, kind="ExternalInput")
    aux_d = nc.dram_tensor("aux", [P, 8], dt.float32, kind="ExternalInput")
    out_d = nc.dram_tensor("out", [C, NQ], dt.float32, kind="ExternalOutput")

    out_ap = out_d.ap().rearrange("(t p) n -> p t n", p=P)

    with tile.TileContext(nc) as tc:
        with (
            nc.allow_low_precision(reason="fp8 attention is intentional"),
            tc.tile_pool(name="persist", bufs=1) as pe_,
            tc.tile_pool(name="pt", bufs=6) as ptp,
            tc.tile_pool(name="tmp", bufs=3) as tmp,
            tc.tile_pool(name="mm", bufs=2, space="PSUM") as mmp,
            tc.tile_pool(name="acc", bufs=2, space="PSUM") as accp,
            tc.tile_pool(name="zp", bufs=2, space="PSUM") as zpp,
        ):
            # ---------- DMAs first: queue engines must trigger before any
            # other work lands on them (first transfer has ~3.5us ramp) ----
            # ksb and xs8 are chunk-major [P, ck, t, 512] so every transfer
            # is fully contiguous (strided multi-segment DMAs ran ~2x slower)
            ksb = pe_.tile([P, 8, CT, 512], dt.float8e4, tag="ksb")
            ksb_flat = ksb.rearrange("p k t n -> p (k t n)")
            xs8 = pe_.tile([P, 4, CT, 512], dt.float8e4, tag="xs8")
            xs8_flat = xs8.rearrange("p k t n -> p (k t n)")
            vT = pe_.tile([P, 32, C], dt.float8e4, tag="vT")
            vT_flat = vT.rearrange("p j c -> p (j c)")
            aux_sb = pe_.tile([P, 8], dt.float32, tag="aux")
            xq_r = pe_.tile([P, CT, NQ], dt.bfloat16, tag="xq")

            # per-queue DMA throughput is only ~50GB/s — parallel queues are
            # what buys bandwidth. Interleave many small transfers across the
            # three queue engines, ordered by consumption deadline
            # (pair u of chunk 0 runs at ~11.9 + 1.15*u us).
            CKB = CT * 512  # flat elems per ksb/xs8 chunk

            def kdma(eng, ck):
                fs = slice(ck * CKB, (ck + 1) * CKB)
                eng.dma_start(ksb_flat[:, fs], ksb_d.ap()[:, fs])

            def xdma(eng, lo, hi):
                fs = slice(lo * CKB, hi * CKB)
                eng.dma_start(xs8_flat[:, fs], xs_d.ap()[:, fs])

            def vdma(eng, lo, hi):
                fs = slice(lo * C, hi * C)
                eng.dma_start(vT_flat[:, fs], vt_d.ap()[:, fs])

            kdma(nc.sync, 0)
            xdma(nc.scalar, 0, 1)
            nc.gpsimd.dma_start(aux_sb, aux_d.ap())
            kdma(nc.gpsimd, 1)
            kdma(nc.sync, 2)
            vdma(nc.scalar, 0, 6)
            kdma(nc.gpsimd, 3)
            kdma(nc.sync, 4)
            kdma(nc.scalar, 5)
            vdma(nc.gpsimd, 6, 12)
            kdma(nc.sync, 6)
            vdma(nc.scalar, 12, 20)
            vdma(nc.gpsimd, 20, 32)
            kdma(nc.gpsimd, 7)
            xdma(nc.scalar, 1, 4)

            def start_xq_dma():
                nc.gpsimd.dma_start(
                    xq_r.rearrange("p t n -> p (t n)"), xq_d.ap()
                )

            bpp = aux_sb[:, 0:2]
            ebias = aux_sb[:, 2:3]  # -shift

            # ---------- constants + PE warm-up ----------
            warm_w = pe_.tile([P, P], dt.bfloat16, tag="warmw")
            nc.vector.memset(warm_w, 0.0)
            warm_x = pe_.tile([P, 512], dt.bfloat16, tag="warmx")
            nc.vector.memset(warm_x, 0.0)
            # pair-dim byte stride must be %16 for DoubleRow ldweights
            ones2_t = pe_.tile([P, 2, 16], dt.float8e5, tag="ones2")
            nc.vector.memset(ones2_t.rearrange("p a b -> p (a b)"), 1.0)
            ones2 = ones2_t[:, :, 0:1]
            ones_row = pe_.tile([1, P], dt.bfloat16, tag="ones1r")
            nc.vector.memset(ones_row, 1.0)
            tjunk = pe_.tile([1, 2], dt.float32, tag="tjunk")
            nc.vector.memset(tjunk, 1.0)
            nc.scalar.activation(tjunk, tjunk, AF.Exp)
            for _ in range(WARM_MMS):
                wps = mmp.tile([P, 2, 512], dt.float32, tag="mm")
                nc.tensor.matmul(wps[:, 0], warm_w, warm_x, start=True, stop=True)

            xb = pe_.tile([P, CT, NQ], dt.bfloat16, tag="xb")

            # ---------- attention: flat pair pipeline across chunks ----------
            NIC = NQ // 512
            NU = 16  # jt pairs per chunk
            pend = {}

            def fin_a(ic):
                isl, a_ps, z_ps = pend[ic]
                acp = tmp.tile([P, CT, 512], dt.float32, tag="acp", name=f"acp{ic}")
                for ch in range(CT):
                    nc.vector.tensor_copy(acp[:, ch], a_ps[ch])
                zc = tmp.tile([1, 512], dt.float32, tag="zc", name=f"zc{ic}")
                nc.vector.tensor_scalar_mul(zc, z_ps, WSCALE)
                zb = tmp.tile([P, 2, 512], dt.float32, tag="zb", name=f"zb{ic}")
                nc.gpsimd.partition_broadcast(zb[:, 0], zc)
                nc.vector.reciprocal_approx_fast(zb[:, 1], zb[:, 0])
                pend[ic] = (isl, acp, zb[:, 1])

            def fin_b(ic):
                isl, acp, zr = pend.pop(ic)
                o_sb = tmp.tile([P, CT, 512], dt.float32, tag="o", name=f"o{ic}")
                for h in range(CT):
                    nc.vector.tensor_mul(o_sb[:, h], acp[:, h], zr)
                    nc.vector.tensor_add(o_sb[:, h], o_sb[:, h], xb[:, h, isl])
                nc.sync.dma_start(out_ap[:, :, isl], o_sb)

            def fin_final(ic):
                # exposed tail: broadcast Z with a K=1 matmul (PE is free),
                # then stream the output in 256-wide pieces
                isl, a_ps, z_ps = pend.pop(ic)
                zc = tmp.tile([1, 512], dt.bfloat16, tag="zcf")
                nc.vector.tensor_scalar_mul(zc, z_ps, WSCALE)
                zb_ps = mmp.tile([P, 2, 512], dt.float32, tag="mm")
                nc.tensor.matmul(zb_ps[:, 0], ones_row, zc, start=True, stop=True)
                zr = tmp.tile([P, 512], dt.float32, tag="zrf")
                nc.vector.reciprocal_approx_fast(zr, zb_ps[:, 0])
                o_sb = tmp.tile([P, CT, 512], dt.float32, tag="o", name="ofin")
                dma_engs = [nc.sync, nc.scalar, nc.sync, nc.scalar]
                for q in range(4):
                    h, hq = q // 2, q % 2
                    qs = slice(hq * 256, (hq + 1) * 256)
                    gsl = slice(isl.start + hq * 256, isl.start + (hq + 1) * 256)
                    oq = o_sb[:, h, qs]
                    # DVE does the PSUM-side muls; gpsimd (SBUF-only) chases
                    # with the residual adds so the two pipelines overlap
                    nc.vector.tensor_mul(oq, a_ps[h][:, qs], zr[:, qs])
                    nc.gpsimd.tensor_add(oq, oq, xb[:, h, gsl])
                    dma_engs[q].dma_start(out_ap[:, h, gsl], oq)

            pairs = [(ic, u) for ic in range(NIC) for u in range(NU)]
            isl_of = lambda ic: slice(ic * 512, (ic + 1) * 512)
            a_ps_of = {}
            z_ps_of = {}
            pts = {}

            def st_exp(ic, u):
                st2 = mmp.tile([P, 2, 512], dt.float32, tag="mm")
                for i in range(2):
                    jt = 2 * u + i
                    kck, kcol = jt // 4, (jt % 4) * P
                    nc.tensor.matmul(
                        st2[:, i],
                        ksb[:, kck, :, kcol : kcol + P],
                        xs8[:, ic],
                        start=True, stop=True, perf_mode=DR,
                    )
                pt2 = ptp.tile([P, 2, 512], dt.float8e5, tag="pt")
                nc.scalar.activation(
                    pt2.rearrange("p a b -> p (a b)"),
                    st2.rearrange("p a b -> p (a b)"),
                    AF.Exp, scale=EXP_SCALE, bias=ebias,
                )
                pts[(ic, u)] = pt2

            st_exp(0, 0)
            st_exp(0, 1)
            for idx, (ic, u) in enumerate(pairs):
                if idx + 2 < len(pairs):
                    st_exp(*pairs[idx + 2])
                if u == 0:
                    a_ps_of[ic] = [
                        accp.tile([P, 512], dt.float32, tag="acc", name=f"acc{ic}_{i}")
                        for i in range(CT)
                    ]
                    z_ps_of[ic] = zpp.tile([1, 512], dt.float32, tag="z", name=f"z{ic}")
                a_ps, z_ps = a_ps_of[ic], z_ps_of[ic]
                pt2 = pts.pop((ic, u))
                # last pair: Z first so the epilogue's Z chain starts under
                # the final PV matmuls
                if u == NU - 1:
                    nc.tensor.matmul(
                        z_ps, ones2, pt2, start=False, stop=True,
                        perf_mode=DR,
                    )
                for ch in range(CT):
                    nc.tensor.matmul(
                        a_ps[ch],
                        vT[:, 2 * u : 2 * u + 2, ch * P : (ch + 1) * P],
                        pt2,
                        start=(u == 0), stop=(u == NU - 1),
                        perf_mode=DR,
                    )
                if u < NU - 1:
                    nc.tensor.matmul(
                        z_ps, ones2, pt2,
                        start=(u == 0), stop=False,
                        perf_mode=DR,
                    )
                if ic == 0 and u == 8:
                    start_xq_dma()
                if ic == 0 and u == NU - 1:
                    # xb = x + proj-bias (bf16); lands in chunk 1's DVE lull
                    for h in range(CT):
                        for hf in range(2):
                            hs = slice(hf * 1024, (hf + 1) * 1024)
                            nc.vector.tensor_scalar_add(
                                xb[:, h, hs], xq_r[:, h, hs], bpp[:, h : h + 1]
                            )
                if u == NU - 1:
                    pend[ic] = (isl_of(ic), a_ps, z_ps)
                    if ic < NIC - 1:
                        fin_a(ic)
                    if ic > 0:
                        fin_b(ic - 1)
            fin_final(NIC - 1)

    nc.compile()
    return nc


def _get_nc():
    if "nc" not in _CACHED:
        _CACHED["nc"] = _build()
    return _CACHED["nc"]


def kernel(x, gn_scale, gn_bias, wq, bq, wk, bk, wv, bv, wp, bp, _trace=False, _trace_cores=None):
    try:
        import jax
        if jax.config.jax_compilation_cache_dir is None:
            jax.config.update("jax_compilation_cache_dir", "/tmp/attnblock_jax_cache")
            jax.config.update("jax_persistent_cache_min_compile_time_secs", 1.0)
    except Exception:
        pass
    import ml_dtypes
    from concourse.bass_utils import run_bass_kernel_spmd

    bf16 = ml_dtypes.bfloat16
    e4 = ml_dtypes.float8_e4m3
    nc = _get_nc()
    x = np.asarray(x, np.float32).reshape(B, C, N)

    def to_e4(a):
        return np.clip(a, -224.0, 224.0).astype(e4)

    wq64 = np.asarray(wq, np.float64)
    wk64 = np.asarray(wk, np.float64)
    wv64 = np.asarray(wv, np.float64)
    wp64 = np.asarray(wp, np.float64)
    mmat = (wq64.T @ wk64).astype(np.float32)
    wpv = (wv64.T @ wp64.T).astype(np.float32)
    gsc = np.asarray(gn_scale, np.float64)
    gbi = np.asarray(gn_bias, np.float64)
    bv64 = np.asarray(bv, np.float64)
    bp64 = np.asarray(bp, np.float64)

    # per-batch GroupNorm stats -> folded weights -> host conv eval
    cg = C // NUM_GROUPS
    ksb_b, vt_b, aux_b = [], [], []
    rng = np.random.default_rng(0)
    sq = rng.choice(N, 48, replace=False)
    for b in range(B):
        xb32 = x[b]
        xg = xb32.reshape(NUM_GROUPS, cg, N)
        mean = xg.mean(axis=(1, 2), dtype=np.float64)
        var = xg.var(axis=(1, 2), dtype=np.float64)
        rstd = 1.0 / np.sqrt(var + EPS)
        alpha = np.repeat(rstd, cg) * gsc
        beta = gbi - np.repeat(mean * rstd, cg) * gsc
        A = ((alpha[:, None] * mmat) * alpha[None, :]).astype(np.float32)
        Wsv = (alpha[:, None] * wpv).astype(np.float32)
        bpp = bp64 + wp64 @ (bv64 + wv64 @ beta)
        khat = A @ xb32                      # [C, N]
        vhat = (WSCALE * Wsv).T @ xb32       # [C, N] -> transpose later
        smax = float((khat[:, sq].T @ xb32).max()) / 16.0
        shift = max(3.0, smax + 1.0 - 7.0)
        ksb_b.append(to_e4(WSCALE * khat))
        vt_b.append(to_e4(vhat))
        aux = np.zeros((P, 8), np.float32)
        aux[:, 0] = bpp[:P]
        aux[:, 1] = bpp[P:]
        aux[:, 2] = -shift
        aux_b.append(aux)

    in_maps = []
    for core in range(8):
        b, qh = core // 2, core % 2
        roll = (lambda a: a) if qh == 0 else (
            lambda a: np.concatenate([a[:, NQ:], a[:, :NQ]], axis=1)
        )
        xl = roll(x[b])
        kl = roll(ksb_b[b])
        vl = roll(vt_b[b])          # [C, N] fp8
        # k_sb pack chunk-major [p, ck*CT*512 + t*512 + c]
        kp = np.ascontiguousarray(
            kl.reshape(CT, P, 8, 512).transpose(1, 2, 0, 3).reshape(P, -1)
        )
        # vT pack [p, jt*C + c] with n = jt*128 + p
        vp = np.ascontiguousarray(
            vl.T.reshape(32, P, C).transpose(1, 0, 2).reshape(P, 32 * C)
        )
        xsq = xl[:, :NQ]
        xs = np.ascontiguousarray(
            to_e4(xsq).reshape(CT, P, 4, 512).transpose(1, 2, 0, 3).reshape(P, -1)
        )
        in_maps.append({
            "ksb": kp,
            "vt": vp,
            "xs": xs,
            "xq": np.ascontiguousarray(
                np.concatenate([xsq[:P], xsq[P:]], axis=1)
            ).astype(bf16),
            "aux": aux_b[b],
        })

    last_err = None
    for attempt in range(3):
        try:
            res = run_bass_kernel_spmd(
                nc, in_maps, core_ids=list(range(8)), trace=_trace,
                trace_cores=_trace_cores,
            )
            break
        except Exception as e:  # transient NRT device faults happen rarely
            last_err = e
            import time as _time

            _time.sleep(2.0 * (attempt + 1))
    else:
        raise last_err
    out = np.empty((B, C, N), np.float32)
    for core in range(8):
        b, qh = core // 2, core % 2
        out[b][:, qh * NQ : (qh + 1) * NQ] = res.results[core]["out"]
    if _trace:
        _CACHED["last_results"] = res
    return out.reshape(B, C, H, W)


# revision 40
# speedup vs baseline: 1.0562x; 1.0140x over previous
"""AttnBlock (GroupNorm + single-head spatial self-attention + residual) on
8 Trainium2 NeuronCores — fp8 DoubleRow edition.

Sharding: batch (4) x query-half (2) -> 8 independent shards, one per core.
The host rolls the flattened spatial axis by 2048 for odd cores so each
core's queries are the first 2048 columns of its local x; K/V see all 4096.

Host preprocessing (per batch, standard norm/weight folding):
  - GroupNorm stats -> alpha/beta folded into the conv weights:
    A = diag(alpha) (Wq^T Wk) diag(alpha)  (scores bilinear form, the
    M-trick: per-query affine cancels under softmax, per-key O(mean) term
    dropped), Wsv = diag(alpha) Wv^T Wp^T (V conv with the output
    projection folded in), bpp = bp + Wp(bv + Wv beta).
  - The two 1x1 convs are evaluated host-side in fp32 and shipped as
    single-quantized fp8e4 operands scaled x16: k_sb = fp8(16 A x) and
    vT = fp8(16 x^T Wsv); queries ship twice (fp8e4 for the score matmul,
    bf16 for the residual).

Device pipeline per core — pure attention, every matmul fp8 DoubleRow
(contraction 256 in one pass, 2 fp8 MACs/cell/cycle):
  4 query chunks of 512, flat 2-pair software pipeline across chunks:
    scores st[j,q] = k_sb^T xq8 (pair tiles in 2 PSUM banks),
    P = exp(st/256 - shift) -> fp8e5 pair tiles (ScalarE, per-partition
    bias carries the shift; e5m2 makes overflow impossible),
    PV: a[c,q] += vT pair^T P pair, and Z accumulated on the PE with a
    [128,2,1] ones DoubleRow matmul per pair (partition reductions are
    ~6x cheaper on the PE than on DVE/GpSimd for fp8 operands).
  Epilogue per chunk (DVE+GpSimd): a * 1/(16Z) + (x + bpp), streamed out.

Steady state is ScalarE-bound: exp of a [128,1024] pair costs
(1024+352)/1.2GHz = 1147ns vs 5x215ns = 1075ns of PE matmul per pair.
"""
import numpy as np

B, C, H, W = 4, 256, 64, 64
N = H * W            # 4096 spatial positions
NQ = N // 2          # 2048 queries per core
P = 128              # partitions
CT = C // P          # 2 channel tiles
NUM_GROUPS = 8
EPS = 1e-5
WSCALE = 16.0        # fp8 operand prescale
EXP_SCALE = 1.0 / 256.0   # score descale: 1/16 (attn) * 1/16 (WSCALE)
WARM_MMS = 6

_CACHED = {}


def _build():
    import concourse.bass as bass
    import concourse.mybir as mybir
    import concourse.tile as tile
    from concourse import bacc

    dt = mybir.dt
    AF = mybir.ActivationFunctionType
    DR = mybir.MatmulPerfMode.DoubleRow

    nc = bacc.Bacc("TRN2", debug=False, num_devices=8)

    ksb_d = nc.dram_tensor("ksb", [P, CT * N], dt.float8e4, kind="ExternalInput")
    vt_d = nc.dram_tensor("vt", [P, 32 * C], dt.float8e4, kind="ExternalInput")
    xs_d = nc.dram_tensor("xs", [P, CT * NQ], dt.float8e4, kind="ExternalInput")
    xq_d = nc.dram_tensor("xq", [P, CT * NQ], dt.bfloat16, kind="ExternalInput")
    aux_d = nc.dram_tensor("aux", [P, 8], dt.float32, kind="ExternalInput")
    out_d = nc.dram_tensor("out", [C, NQ], dt.float32, kind="ExternalOutput")

    out_ap = out_d.ap().rearrange("(t p) n -> p t n", p=P)

    with tile.TileContext(nc) as tc:
        with (
            nc.allow_low_precision(reason="fp8 attention is intentional"),
            tc.tile_pool(name="persist", bufs=1) as pe_,
            tc.tile_pool(name="pt", bufs=6) as ptp,
            tc.tile_pool(name="tmp", bufs=3) as tmp,
            tc.tile_pool(name="mm", bufs=2, space="PSUM") as mmp,
            tc.tile_pool(name="acc", bufs=2, space="PSUM") as accp,
            tc.tile_pool(name="zp", bufs=2, space="PSUM") as zpp,
        ):
            # ---------- DMAs first: queue engines must trigger before any
            # other work lands on them (first transfer has ~3.5us ramp) ----
            # ksb and xs8 are chunk-major [P, ck, t, 512] so every transfer
            # is fully contiguous (strided multi-segment DMAs ran ~2x slower)
            ksb = pe_.tile([P, 8, CT, 512], dt.float8e4, tag="ksb")
            ksb_flat = ksb.rearrange("p k t n -> p (k t n)")
            xs8 = pe_.tile([P, 4, CT, 512], dt.float8e4, tag="xs8")
            xs8_flat = xs8.rearrange("p k t n -> p (k t n)")
            vT = pe_.tile([P, 32, C], dt.float8e4, tag="vT")
            vT_flat = vT.rearrange("p j c -> p (j c)")
            aux_sb = pe_.tile([P, 8], dt.float32, tag="aux")
            xq_r = pe_.tile([P, CT, NQ], dt.bfloat16, tag="xq")

            # per-queue DMA throughput is only ~50GB/s — parallel queues are
            # what buys bandwidth. Interleave many small transfers across the
            # three queue engines, ordered by consumption deadline
            # (pair u of chunk 0 runs at ~11.9 + 1.15*u us).
            CKB = CT * 512  # flat elems per ksb/xs8 chunk

            def kdma(eng, ck):
                fs = slice(ck * CKB, (ck + 1) * CKB)
                eng.dma_start(ksb_flat[:, fs], ksb_d.ap()[:, fs])

            def xdma(eng, lo, hi):
                fs = slice(lo * CKB, hi * CKB)
                eng.dma_start(xs8_flat[:, fs], xs_d.ap()[:, fs])

            def vdma(eng, lo, hi):
                fs = slice(lo * C, hi * C)
                eng.dma_start(vT_flat[:, fs], vt_d.ap()[:, fs])

            kdma(nc.sync, 0)
            xdma(nc.scalar, 0, 1)
            nc.gpsimd.dma_start(aux_sb, aux_d.ap())
            kdma(nc.gpsimd, 1)
            kdma(nc.sync, 2)
            vdma(nc.scalar, 0, 6)
            kdma(nc.gpsimd, 3)
            kdma(nc.sync, 4)
            kdma(nc.scalar, 5)
            vdma(nc.gpsimd, 6, 12)
            kdma(nc.sync, 6)
            vdma(nc.scalar, 12, 20)
            vdma(nc.gpsimd, 20, 32)
            kdma(nc.gpsimd, 7)
            xdma(nc.scalar, 1, 4)

            def start_xq_dma():
                nc.gpsimd.dma_start(
                    xq_r.rearrange("p t n -> p (t n)"), xq_d.ap()
                )

            bpp = aux_sb[:, 0:2]
            ebias = aux_sb[:, 2:3]  # -shift

            # ---------- constants + PE warm-up ----------
            warm_w = pe_.tile([P, P], dt.bfloat16, tag="warmw")
            nc.vector.memset(warm_w, 0.0)
            warm_x = pe_.tile([P, 512], dt.bfloat16, tag="warmx")
            nc.vector.memset(warm_x, 0.0)
            # pair-dim byte stride must be %16 for DoubleRow ldweights
            ones2_t = pe_.tile([P, 2, 16], dt.float8e5, tag="ones2")
            nc.vector.memset(ones2_t.rearrange("p a b -> p (a b)"), 1.0)
            ones2 = ones2_t[:, :, 0:1]
            ones_row = pe_.tile([1, P], dt.bfloat16, tag="ones1r")
            nc.vector.memset(ones_row, 1.0)
            tjunk = pe_.tile([1, 2], dt.float32, tag="tjunk")
            nc.vector.memset(tjunk, 1.0)
            nc.scalar.activation(tjunk, tjunk, AF.Exp)
            for _ in range(WARM_MMS):
                wps = mmp.tile([P, 2, 512], dt.float32, tag="mm")
                nc.tensor.matmul(wps[:, 0], warm_w, warm_x, start=True, stop=True)

            xb = pe_.tile([P, CT, NQ], dt.bfloat16, tag="xb")

            # ---------- attention: flat pair pipeline across chunks ----------
            NIC = NQ // 512
            NU = 16  # jt pairs per chunk
            pend = {}

            def fin_a(ic):
                isl, a_ps, z_ps = pend[ic]
                acp = tmp.tile([P, CT, 512], dt.float32, tag="acp", name=f"acp{ic}")
                for ch in range(CT):
                    nc.vector.tensor_copy(acp[:, ch], a_ps[ch])
                zc = tmp.tile([1, 512], dt.float32, tag="zc", name=f"zc{ic}")
                nc.vector.tensor_scalar_mul(zc, z_ps, WSCALE)
                zb = tmp.tile([P, 2, 512], dt.float32, tag="zb", name=f"zb{ic}")
                nc.gpsimd.partition_broadcast(zb[:, 0], zc)
                nc.vector.reciprocal_approx_fast(zb[:, 1], zb[:, 0])
                pend[ic] = (isl, acp, zb[:, 1])

            def fin_b(ic):
                isl, acp, zr = pend.pop(ic)
                o_sb = tmp.tile([P, CT, 512], dt.float32, tag="o", name=f"o{ic}")
                for h in range(CT):
                    nc.vector.tensor_mul(o_sb[:, h], acp[:, h], zr)
                    nc.vector.tensor_add(o_sb[:, h], o_sb[:, h], xb[:, h, isl])
                nc.sync.dma_start(out_ap[:, :, isl], o_sb)

            def fin_final(ic):
                # exposed tail: broadcast Z with a K=1 matmul (PE is free),
                # then stream the output in 256-wide pieces
                isl, a_ps, z_ps = pend.pop(ic)
                zc = tmp.tile([1, 512], dt.bfloat16, tag="zcf")
                nc.vector.tensor_scalar_mul(zc, z_ps, WSCALE)
                zb_ps = mmp.tile([P, 2, 512], dt.float32, tag="mm")
                nc.tensor.matmul(zb_ps[:, 0], ones_row, zc, start=True, stop=True)
                zr = tmp.tile([P, 512], dt.float32, tag="zrf")
                nc.vector.reciprocal_approx_fast(zr, zb_ps[:, 0])
                o_sb = tmp.tile([P, CT, 512], dt.float32, tag="o", name="ofin")
                dma_engs = [nc.sync, nc.scalar, nc.sync, nc.scalar]
                for q in range(4):
                    h, hq = q // 2, q % 2
                    qs = slice(hq * 256, (hq + 1) * 256)
                    gsl = slice(isl.start + hq * 256, isl.start + (hq + 1) * 256)
                    oq = o_sb[:, h, qs]
                    # DVE does the PSUM-side muls; gpsimd (SBUF-only) chases
                    # with the residual adds so the two pipelines overlap
                    nc.vector.tensor_mul(oq, a_ps[h][:, qs], zr[:, qs])
                    nc.gpsimd.tensor_add(oq, oq, xb[:, h, gsl])
                    dma_engs[q].dma_start(out_ap[:, h, gsl], oq)

            pairs = [(ic, u) for ic in range(NIC) for u in range(NU)]
            isl_of = lambda ic: slice(ic * 512, (ic + 1) * 512)
            a_ps_of = {}
            z_ps_of = {}
            pts = {}

            def st_exp(ic, u):
                st2 = mmp.tile([P, 2, 512], dt.float32, tag="mm")
                for i in range(2):
                    jt = 2 * u + i
                    kck, kcol = jt // 4, (jt % 4) * P
                    nc.tensor.matmul(
                        st2[:, i],
                        ksb[:, kck, :, kcol : kcol + P],
                        xs8[:, ic],
                        start=True, stop=True, perf_mode=DR,
                    )
                pt2 = ptp.tile([P, 2, 512], dt.float8e5, tag="pt")
                nc.scalar.activation(
                    pt2.rearrange("p a b -> p (a b)"),
                    st2.rearrange("p a b -> p (a b)"),
                    AF.Exp, scale=EXP_SCALE, bias=ebias,
                )
                pts[(ic, u)] = pt2

            st_exp(0, 0)
            st_exp(0, 1)
            for idx, (ic, u) in enumerate(pairs):
                if idx + 2 < len(pairs):
                    st_exp(*pairs[idx + 2])
                if u == 0:
                    a_ps_of[ic] = [
                        accp.tile([P, 512], dt.float32, tag="acc", name=f"acc{ic}_{i}")
                        for i in range(CT)
                    ]
                    z_ps_of[ic] = zpp.tile([1, 512], dt.float32, tag="z", name=f"z{ic}")
                a_ps, z_ps = a_ps_of[ic], z_ps_of[ic]
                pt2 = pts.pop((ic, u))
                # last pair: Z first so the epilogue's Z chain starts under
                # the final PV matmuls
                if u == NU - 1:
                    nc.tensor.matmul(
                        z_ps, ones2, pt2, start=False, stop=True,
                        perf_mode=DR,
                    )
                for ch in range(CT):
                    nc.tensor.matmul(
                        a_ps[ch],
                        vT[:, 2 * u : 2 * u + 2, ch * P : (ch + 1) * P],
                        pt2,
                        start=(u == 0), stop=(u == NU - 1),
                        perf_mode=DR,
                    )
                if u < NU - 1:
                    nc.tensor.matmul(
                        z_ps, ones2, pt2,
                        start=(u == 0), stop=False,
                        perf_mode=DR,
                    )
                if ic == 0 and u == 8:
                    start_xq_dma()
                if ic == 0 and u == NU - 1:
                    # xb = x + proj-bias (bf16); lands in chunk 1's DVE lull
                    for h in range(CT):
                        for hf in range(2):
                            hs = slice(hf * 1024, (hf + 1) * 1024)
                            nc.vector.tensor_scalar_add(
                                xb[:, h, hs], xq_r[:, h, hs], bpp[:, h : h + 1]
                            )
                if u == NU - 1:
                    pend[ic] = (isl_of(ic), a_ps, z_ps)
                    if ic < NIC - 1:
                        fin_a(ic)
                    if ic > 0:
                        fin_b(ic - 1)
            fin_final(NIC - 1)

    nc.compile()
    return nc


def _get_nc():
    if "nc" not in _CACHED:
        _CACHED["nc"] = _build()
    return _CACHED["nc"]


def kernel(x, gn_scale, gn_bias, wq, bq, wk, bk, wv, bv, wp, bp, _trace=False, _trace_cores=None):
    try:
        import jax
        if jax.config.jax_compilation_cache_dir is None:
            jax.config.update("jax_compilation_cache_dir", "/tmp/attnblock_jax_cache")
            jax.config.update("jax_persistent_cache_min_compile_time_secs", 1.0)
    except Exception:
        pass
    import ml_dtypes
    from concourse.bass_utils import run_bass_kernel_spmd

    bf16 = ml_dtypes.bfloat16
    e4 = ml_dtypes.float8_e4m3
    nc = _get_nc()
    x = np.asarray(x, np.float32).reshape(B, C, N)

    def to_e4(a):
        return np.clip(a, -224.0, 224.0).astype(e4)

    wq64 = np.asarray(wq, np.float64)
    wk64 = np.asarray(wk, np.float64)
    wv64 = np.asarray(wv, np.float64)
    wp64 = np.asarray(wp, np.float64)
    mmat = (wq64.T @ wk64).astype(np.float32)
    wpv = (wv64.T @ wp64.T).astype(np.float32)
    gsc = np.asarray(gn_scale, np.float64)
    gbi = np.asarray(gn_bias, np.float64)
    bv64 = np.asarray(bv, np.float64)
    bp64 = np.asarray(bp, np.float64)

    # per-batch GroupNorm stats -> folded weights -> host conv eval
    cg = C // NUM_GROUPS
    ksb_b, vt_b, aux_b = [], [], []
    rng = np.random.default_rng(0)
    sq = rng.choice(N, 48, replace=False)
    for b in range(B):
        xb32 = x[b]
        xg = xb32.reshape(NUM_GROUPS, cg, N)
        mean = xg.mean(axis=(1, 2), dtype=np.float64)
        var = xg.var(axis=(1, 2), dtype=np.float64)
        rstd = 1.0 / np.sqrt(var + EPS)
        alpha = np.repeat(rstd, cg) * gsc
        beta = gbi - np.repeat(mean * rstd, cg) * gsc
        A = ((alpha[:, None] * mmat) * alpha[None, :]).astype(np.float32)
        Wsv = (alpha[:, None] * wpv).astype(np.float32)
        bpp = bp64 + wp64 @ (bv64 + wv64 @ beta)
        khat = A @ xb32                      # [C, N]
        vhat = (WSCALE * Wsv).T @ xb32       # [C, N] -> transpose later
        smax = float((khat[:, sq].T @ xb32).max()) / 16.0
        shift = max(3.0, smax + 1.0 - 7.0)
        ksb_b.append(to_e4(WSCALE * khat))
        vt_b.append(to_e4(vhat))
        aux = np.zeros((P, 8), np.float32)
        aux[:, 0] = bpp[:P]
        aux[:, 1] = bpp[P:]
        aux[:, 2] = -shift
        aux_b.append(aux)

    in_maps = []
    for core in range(8):
        b, qh = core // 2, core % 2
        roll = (lambda a: a) if qh == 0 else (
            lambda a: np.concatenate([a[:, NQ:], a[:, :NQ]], axis=1)
        )
        xl = roll(x[b])
        kl = roll(ksb_b[b])
        vl = roll(vt_b[b])          # [C, N] fp8
        # k_sb pack chunk-major [p, ck*CT*512 + t*512 + c]
        kp = np.ascontiguousarray(
            kl.reshape(CT, P, 8, 512).transpose(1, 2, 0, 3).reshape(P, -1)
        )
        # vT pack [p, jt*C + c] with n = jt*128 + p
        vp = np.ascontiguousarray(
            vl.T.reshape(32, P, C).transpose(1, 0, 2).reshape(P, 32 * C)
        )
        xsq = xl[:, :NQ]
        xs = np.ascontiguousarray(
            to_e4(xsq).reshape(CT, P, 4, 512).transpose(1, 2, 0, 3).reshape(P, -1)
        )
        in_maps.append({
            "ksb": kp,
            "vt": vp,
            "xs": xs,
            "xq": np.ascontiguousarray(
                np.concatenate([xsq[:P], xsq[P:]], axis=1)
            ).astype(bf16),
            "aux": aux_b[b],
        })

    last_err = None
    for attempt in range(3):
        try:
            res = run_bass_kernel_spmd(
                nc, in_maps, core_ids=list(range(8)), trace=_trace,
                trace_cores=_trace_cores,
            )
            break
        except Exception as e:  # transient NRT device faults happen rarely
            last_err = e
            import time as _time

            _time.sleep(2.0 * (attempt + 1))
    else:
        raise last_err
    out = np.empty((B, C, N), np.float32)
    for core in range(8):
        b, qh = core // 2, core % 2
        out[b][:, qh * NQ : (qh + 1) * NQ] = res.results[core]["out"]
    if _trace:
        _CACHED["last_results"] = res
    return out.reshape(B, C, H, W)
